# revision 1
# baseline (speedup 1.0000x reference)
"""Multi-head attention Trainium2 kernel (8-core SPMD, no collectives).

Sharding: 8 cores = 4 batches x 2 sequence-halves. Each core receives the
full x[b] (to compute K/V over all S keys) plus its own half of the query
rows, and produces a disjoint [NQ, D] slice of the output. K/V projection
work is duplicated across the 2 cores sharing a batch (~25% extra PE work)
in exchange for zero inter-core communication.

Per-core pipeline (all matmul inputs bf16, PSUM accumulation fp32):
  A1: V = x @ Wv               -> VP [keys, h, 65] with a ones column at 64
  A2: K^T, Q^T per head-pair   -> KT/QT [2h*64, S]
  B:  S^T[k,q] = K^T.T Q^T; P = exp(0.125*S^T); acc[q, 65] += P^T V'
      (col 64 of acc = softmax denominator); normalize by reciprocal;
      transpose back to [hv, q] -> CT
  C:  out[q,:] = CT.T @ Wo + ones.T @ bo
Softmax skips max-subtraction: scores ~ N(0,1) with |s|<~10, exp is safe.
"""

import numpy as np
import ml_dtypes

import concourse.bass as bass
import concourse.bacc as bacc
import concourse.mybir as mybir
import concourse.tile as tile
from concourse import masks

B, S, D = 4, 2048, 1024
H, DQ, DV = 16, 64, 64
P = 128
NQ = S // 2            # query rows per core
NPAIR = H // 2         # head pairs (2 heads packed on 128 partitions)
NDC = D // P           # 8 contraction chunks of D
NKC = S // P           # 16 key blocks
NQB = NQ // P          # 8 query blocks
NCORES = 8
BF16 = mybir.dt.bfloat16
F32 = mybir.dt.float32


def build_nc(reps=1):
    nc = bacc.Bacc("TRN2", target_bir_lowering=False, debug=False,
                   num_devices=NCORES)

    # Host supplies partition-major layouts (see kernel() below).
    xT = nc.dram_tensor("xT", [P, NDC, S], BF16, kind="ExternalInput")
    wk = nc.dram_tensor("wk", [NPAIR, P, NDC, P], BF16, kind="ExternalInput")
    wq = nc.dram_tensor("wq", [NPAIR, P, NDC, P], BF16, kind="ExternalInput")
    wv = nc.dram_tensor("wv", [P, NDC, H * DV], BF16, kind="ExternalInput")
    wo = nc.dram_tensor("wo", [P, NPAIR, D], BF16, kind="ExternalInput")
    bo = nc.dram_tensor("bo", [1, D], BF16, kind="ExternalInput")
    out = nc.dram_tensor("out", [NQ, D], F32, kind="ExternalOutput")

    Exp = mybir.ActivationFunctionType.Exp

    with tile.TileContext(nc) as tc:
      for _rep in range(reps):
        with (
            tc.tile_pool(name="const", bufs=1) as constp,
            tc.tile_pool(name="persist", bufs=1) as persist,
        ):
            bo_sb = constp.tile([1, D], BF16, tag="bo")
            nc.sync.dma_start(bo_sb[:], bo[:])
            bo_bc = constp.tile([P, D], BF16, tag="bo_bc")
            nc.gpsimd.partition_broadcast(bo_bc[:], bo_sb[:])

            KT = persist.tile([P, NPAIR, S], BF16, tag="KT")
            QT = persist.tile([P, NPAIR, NQ], BF16, tag="QT")
            VP = persist.tile([P, NKC, H, DV + 1], BF16, tag="VP")
            CT = persist.tile([P, NPAIR, NQ], BF16, tag="CT")

            # ones column of V' (softmax denominator accumulator)
            nc.vector.memset(VP[:, :, :, DV:DV + 1], 1.0)

            # prefetch the output-projection weights so phase C never waits
            wot = persist.tile([P, NPAIR, D], BF16, tag="wo")
            for pc in range(NPAIR):
                nc.sync.dma_start(wot[:, pc, :], wo[:, pc, :])

            with tc.tile_pool(name="xw", bufs=1) as xw:
                xt = xw.tile([P, NDC, S], BF16, tag="xt")
                wvt = xw.tile([P, NDC, H * DV], BF16, tag="wv")
                for dc in range(NDC):
                    nc.sync.dma_start(xt[:, dc, :], xT[:, dc, :])
                    nc.sync.dma_start(wvt[:, dc, :], wv[:, dc, :])

                # ---- A1: V projection, all heads at once ----
                with tc.tile_pool(name="psA1", bufs=4,
                                  space=bass.MemorySpace.PSUM) as psA1:
                    for sb in range(NKC):
                        vps = psA1.tile([P, H * DV], F32, tag="v")
                        for dc in range(NDC):
                            for half in range(2):
                                nc.tensor.matmul(
                                    vps[:, half * 512:(half + 1) * 512],
                                    xt[:, dc, sb * P:(sb + 1) * P],
                                    wvt[:, dc, half * 512:(half + 1) * 512],
                                    start=(dc == 0), stop=(dc == NDC - 1))
                        nc.vector.tensor_copy(
                            VP[:, sb, :, 0:DV],
                            vps[:].rearrange("p (h v) -> p h v", h=H))

                # ---- A2 + B interleaved: per head pair, project K^T/Q^T
                # then run both heads' attention. The K/Q projection matmuls
                # of pair p+1 fill the PE gaps while pair p's attention is
                # paced by the ACT engine's exp throughput.
                with (
                    tc.tile_pool(name="wkq", bufs=2) as wkq,
                    tc.tile_pool(name="ptp", bufs=3) as ptp,
                    tc.tile_pool(name="nrm", bufs=4) as nrmp,
                    tc.tile_pool(name="psAB", bufs=2,
                                 space=bass.MemorySpace.PSUM) as psAB,
                ):
                    for pair in range(NPAIR):
                        wkp = wkq.tile([P, NDC, P], BF16, tag="wk")
                        nc.sync.dma_start(wkp[:], wk[pair])
                        wqp = wkq.tile([P, NDC, P], BF16, tag="wq")
                        nc.sync.dma_start(wqp[:], wq[pair])

                        # K^T: 4 sequential one-bank psum tiles
                        for nb in range(4):
                            kps = psAB.tile([P, 512], F32, tag="kq")
                            for dc in range(NDC):
                                nc.tensor.matmul(
                                    kps[:],
                                    wkp[:, dc, :],
                                    xt[:, dc, nb * 512:(nb + 1) * 512],
                                    start=(dc == 0), stop=(dc == NDC - 1))
                            nc.vector.tensor_copy(
                                KT[:, pair, nb * 512:(nb + 1) * 512], kps[:])
                        for nb in range(2):
                            qps = psAB.tile([P, 512], F32, tag="kq")
                            for dc in range(NDC):
                                nc.tensor.matmul(
                                    qps[:],
                                    wqp[:, dc, :],
                                    xt[:, dc, nb * 512:(nb + 1) * 512],
                                    start=(dc == 0), stop=(dc == NDC - 1))
                            nc.vector.tensor_copy(
                                QT[:, pair, nb * 512:(nb + 1) * 512], qps[:])

                        for h in (2 * pair, 2 * pair + 1):
                            hh = (h % 2) * 64
                            # acc[g]: out_h^T for queries [g*512,(g+1)*512):
                            # rows 0..63 = sum_k P[k,q]V[k,v]; row 64 = denom
                            accs = [psAB.tile([DV + 1, 512], F32, tag="acc",
                                              name=f"acc{g}") for g in range(2)]
                            for kc in range(NKC):
                                sc = psAB.tile([P, NQ], F32, tag="sc")
                                for half in range(2):
                                    nc.tensor.matmul(
                                        sc[:, half * 512:(half + 1) * 512],
                                        KT[hh:hh + 64, pair,
                                           kc * P:(kc + 1) * P],
                                        QT[hh:hh + 64, pair,
                                           half * 512:(half + 1) * 512],
                                        start=True, stop=True)
                                pt = ptp.tile([P, NQ], BF16, tag="pt")
                                nc.scalar.activation(pt[:], sc[:], Exp,
                                                     scale=0.125)
                                for g in range(2):
                                    nc.tensor.matmul(
                                        accs[g][:],
                                        VP[:, kc, h, :],
                                        pt[:, g * 512:(g + 1) * 512],
                                        start=(kc == 0), stop=(kc == NKC - 1))
                            for g in range(2):
                                rec = nrmp.tile([1, 512], F32, tag="rec")
                                nc.vector.reciprocal(
                                    rec[:], accs[g][DV:DV + 1, :])
                                bc = nrmp.tile([DV, 512], F32, tag="bc")
                                nc.gpsimd.partition_broadcast(bc[:], rec[:])
                                nc.vector.tensor_mul(
                                    CT[hh:hh + 64, pair,
                                       g * 512:(g + 1) * 512],
                                    accs[g][0:DV, :], bc[:])

            # ---- C: output projection + bias ----
            with (
                tc.tile_pool(name="outp", bufs=2) as outp,
                tc.tile_pool(name="psC", bufs=2,
                             space=bass.MemorySpace.PSUM) as psC,
            ):
                for qb in range(NQB):
                    ops = psC.tile([P, D], F32, tag="o")
                    for pc in range(NPAIR):
                        for half in range(2):
                            nc.tensor.matmul(
                                ops[:, half * 512:(half + 1) * 512],
                                CT[:, pc, qb * P:(qb + 1) * P],
                                wot[:, pc, half * 512:(half + 1) * 512],
                                start=(pc == 0), stop=(pc == NPAIR - 1))
                    outsb = outp.tile([P, D], F32, tag="out")
                    nc.vector.tensor_add(outsb[:], ops[:], bo_bc[:])
                    nc.sync.dma_start(out[qb * P:(qb + 1) * P, :], outsb[:])

    nc.compile()
    return nc


def make_in_maps(x, Wq, Wk, Wv, Wo, bo):
    bf = ml_dtypes.bfloat16
    x = np.asarray(x, np.float32)

    def pm(a):  # [D, N] -> partition-major [P, NDC, N]
        return np.ascontiguousarray(
            a.reshape(NDC, P, a.shape[1]).transpose(1, 0, 2)).astype(bf)

    def wpairs(W):  # [H, D, 64] -> [NPAIR, P, NDC, P]
        a = np.asarray(W, np.float32).transpose(1, 0, 2).reshape(D, H * 64)
        return np.ascontiguousarray(
            a.reshape(NDC, P, NPAIR, P).transpose(2, 1, 0, 3)).astype(bf)

    wk_h = wpairs(Wk)
    wq_h = wpairs(Wq)
    wv_h = pm(np.asarray(Wv, np.float32).transpose(1, 0, 2).reshape(D, H * DV))
    wo_h = np.ascontiguousarray(
        np.asarray(Wo, np.float32).reshape(NPAIR, P, D)
        .transpose(1, 0, 2)).astype(bf)
    bo_h = np.asarray(bo, np.float32).reshape(1, D).astype(bf)

    in_maps = []
    for c in range(NCORES):
        b, qs = c // 2, (c % 2) * NQ
        # Sequence permuted so this core's query half occupies columns
        # 0:NQ (attention is invariant to key order, so K/V computed on
        # the permuted sequence give identical outputs).
        xp = np.concatenate([x[b, qs:qs + NQ], x[b, NQ - qs:2 * NQ - qs]], 0)
        in_maps.append({
            "xT": pm(np.ascontiguousarray(xp.T)),
            "wk": wk_h, "wq": wq_h, "wv": wv_h, "wo": wo_h, "bo": bo_h,
        })
    return in_maps


def kernel(x, Wq, Wk, Wv, Wo, bo):
    from concourse.bass_utils import run_bass_kernel_spmd
    in_maps = make_in_maps(x, Wq, Wk, Wv, Wo, bo)
    nc = build_nc()
    res = run_bass_kernel_spmd(nc, in_maps, list(range(NCORES))).results
    full = np.empty((B, S, D), np.float32)
    for c in range(NCORES):
        b, qs = c // 2, (c % 2) * NQ
        full[b, qs:qs + NQ, :] = np.asarray(res[c]["out"], np.float32)
    return full



# revision 2
# speedup vs baseline: 1.0240x; 1.0240x over previous
"""Multi-head attention Trainium2 kernel (8-core SPMD, no collectives). v2

Sharding: 8 cores = 4 batches x 2 sequence-halves (same as v1). Each core
receives the full x[b] (to compute K/V over all S keys) plus its own half of
the query rows, and produces a disjoint [NQ, D] slice of the output.

v2 restructures the attention phase to keep the PE busy continuously:
  - B-phase processes queries in 512-blocks (g) with BOTH heads of a pair
    interleaved, so the two heads' contraction-64 scores matmuls sit on
    PE row-tiles T0/T8 back-to-back.
  - attn@V matmuls are software-pipelined one key-block behind the scores,
    hiding the ACT-engine exp latency.
  - The next pair's K^T/Q^T projection matmuls are injected into the PE
    gaps of the (ACT-paced) attention loop.
PSUM budget during B: 4 sc + 2 acc + 2 inject = 8 banks.

Per-core pipeline (all matmul inputs bf16, PSUM accumulation fp32):
  A1: V = x @ Wv               -> VP [keys, h, 65] with a ones column at 64
  A2: K^T, Q^T per head-pair   -> KT/QT [2h*64, S]   (pair 0 serial; pairs
      1..7 injected into the previous pair's B loop)
  B:  S^T[k,q] = K^T.T Q^T; P = exp(0.125*S^T); acc[q, 65] += P^T V'
      (col 64 of acc = softmax denominator); normalize by reciprocal
  C:  out[q,:] = CT.T @ Wo + bo
Softmax skips max-subtraction: scores ~ N(0,1) with |s|<~10, exp is safe.
"""

import numpy as np
import ml_dtypes

import concourse.bass as bass
import concourse.bacc as bacc
import concourse.mybir as mybir
import concourse.tile as tile

B, S, D = 4, 2048, 1024
H, DQ, DV = 16, 64, 64
P = 128
NQ = S // 2            # query rows per core
NPAIR = H // 2         # head pairs (2 heads packed on 128 partitions)
NDC = D // P           # 8 contraction chunks of D
NKC = S // P           # 16 key blocks
NQB = NQ // P          # 8 query blocks
NG = NQ // 512         # 2 query groups of 512
NCORES = 8
BF16 = mybir.dt.bfloat16
F32 = mybir.dt.float32


def build_nc(reps=1):
    nc = bacc.Bacc("TRN2", target_bir_lowering=False, debug=False,
                   num_devices=NCORES)

    # Host supplies partition-major layouts (see kernel() below).
    xT = nc.dram_tensor("xT", [P, NDC, S], BF16, kind="ExternalInput")
    wk = nc.dram_tensor("wk", [NPAIR, P, NDC, P], BF16, kind="ExternalInput")
    wq = nc.dram_tensor("wq", [NPAIR, P, NDC, P], BF16, kind="ExternalInput")
    wv = nc.dram_tensor("wv", [P, NDC, H * DV], BF16, kind="ExternalInput")
    wo = nc.dram_tensor("wo", [P, NPAIR, D], BF16, kind="ExternalInput")
    bo = nc.dram_tensor("bo", [1, D], BF16, kind="ExternalInput")
    out = nc.dram_tensor("out", [NQ, D], F32, kind="ExternalOutput")

    Exp = mybir.ActivationFunctionType.Exp

    with tile.TileContext(nc) as tc:
      for _rep in range(reps):
        with (
            tc.tile_pool(name="const", bufs=1) as constp,
            tc.tile_pool(name="persist", bufs=1) as persist,
        ):
            bo_sb = constp.tile([1, D], BF16, tag="bo")
            nc.sync.dma_start(bo_sb[:], bo[:])
            bo_bc = constp.tile([P, D], BF16, tag="bo_bc")
            nc.gpsimd.partition_broadcast(bo_bc[:], bo_sb[:])

            KT = persist.tile([P, NPAIR, S], BF16, tag="KT")
            QT = persist.tile([P, NPAIR, NQ], BF16, tag="QT")
            VP = persist.tile([P, NKC, H, DV + 1], BF16, tag="VP")
            CT = persist.tile([P, NPAIR, NQ], BF16, tag="CT")

            # ones column of V' (softmax denominator accumulator)
            nc.vector.memset(VP[:, :, :, DV:DV + 1], 1.0)

            # prefetch the output-projection weights so phase C never waits
            wot = persist.tile([P, NPAIR, D], BF16, tag="wo")
            for pc in range(NPAIR):
                nc.sync.dma_start(wot[:, pc, :], wo[:, pc, :])

            with tc.tile_pool(name="xw", bufs=1) as xw:
                xt = xw.tile([P, NDC, S], BF16, tag="xt")
                wvt = xw.tile([P, NDC, H * DV], BF16, tag="wv")
                for dc in range(NDC):
                    nc.sync.dma_start(xt[:, dc, :], xT[:, dc, :])
                    nc.sync.dma_start(wvt[:, dc, :], wv[:, dc, :])

                # ---- A1: V projection, all heads at once ----
                with tc.tile_pool(name="psA1", bufs=4,
                                  space=bass.MemorySpace.PSUM) as psA1:
                    for sb in range(NKC):
                        vps = psA1.tile([P, H * DV], F32, tag="v")
                        for dc in range(NDC):
                            for half in range(2):
                                nc.tensor.matmul(
                                    vps[:, half * 512:(half + 1) * 512],
                                    xt[:, dc, sb * P:(sb + 1) * P],
                                    wvt[:, dc, half * 512:(half + 1) * 512],
                                    start=(dc == 0), stop=(dc == NDC - 1))
                        nc.vector.tensor_copy(
                            VP[:, sb, :, 0:DV],
                            vps[:].rearrange("p (h v) -> p h v", h=H))

                # ---- A2 + B: pair 0's K^T/Q^T projected serially; pair
                # p+1's projection matmuls are injected one or two at a
                # time into the PE gaps of pair p's ACT-paced B loop.
                with (
                    tc.tile_pool(name="wkq", bufs=2) as wkq,
                    tc.tile_pool(name="ptp", bufs=4) as ptp,
                    tc.tile_pool(name="nrm", bufs=4) as nrmp,
                    tc.tile_pool(name="scp", bufs=4,
                                 space=bass.MemorySpace.PSUM) as scp,
                    tc.tile_pool(name="accp", bufs=2,
                                 space=bass.MemorySpace.PSUM) as accp,
                    tc.tile_pool(name="injp", bufs=2,
                                 space=bass.MemorySpace.PSUM) as injp,
                ):
                    def load_wkq(pair):
                        wkp = wkq.tile([P, NDC, P], BF16, tag="wk")
                        nc.sync.dma_start(wkp[:], wk[pair])
                        wqp = wkq.tile([P, NDC, P], BF16, tag="wq")
                        nc.sync.dma_start(wqp[:], wq[pair])
                        return wkp, wqp

                    def make_fills(pair, wkp, wqp):
                        """Closures, each emitting one projection matmul for
                        `pair`; the last of each 8-matmul accumulation also
                        emits the psum->SBUF copy."""
                        fills = []
                        seqs = ([(KT, wkp, nb) for nb in range(4)]
                                + [(QT, wqp, nb) for nb in range(2)])
                        for si, (dst, wt, nb) in enumerate(seqs):
                            cell = {}

                            def mk(dst=dst, wt=wt, nb=nb, cell=cell, si=si):
                                def emit(dc):
                                    if dc == 0:
                                        cell["ps"] = injp.tile(
                                            [P, 512], F32, tag="inj",
                                            name=f"inj{pair}_{si}")
                                    nc.tensor.matmul(
                                        cell["ps"][:],
                                        wt[:, dc, :],
                                        xt[:, dc, nb * 512:(nb + 1) * 512],
                                        start=(dc == 0), stop=(dc == NDC - 1))
                                    if dc == NDC - 1:
                                        nc.vector.tensor_copy(
                                            dst[:, pair,
                                                nb * 512:(nb + 1) * 512],
                                            cell["ps"][:])
                                return emit
                            emit = mk()
                            for dc in range(NDC):
                                fills.append(lambda dc=dc, emit=emit:
                                             emit(dc))
                        return fills

                    wkp, wqp = load_wkq(0)
                    for f in make_fills(0, wkp, wqp):
                        f()

                    for pair in range(NPAIR):
                        if pair + 1 < NPAIR:
                            wkp, wqp = load_wkq(pair + 1)
                            fills = make_fills(pair + 1, wkp, wqp)
                        else:
                            fills = []
                        fi = 0
                        it = 0
                        nit = NG * NKC
                        for g in range(NG):
                            q0 = g * 512
                            accs = [accp.tile([DV + 1, 512], F32, tag="acc",
                                              name=f"acc{pair}_{g}_{h}")
                                    for h in range(2)]
                            prev_pt = None
                            for kc in range(NKC):
                                # both heads' scores into one 2-bank psum
                                # tile: adjacent T0/T8 row-tile matmuls,
                                # then a single 1024-col exp
                                sc = scp.tile([P, 1024], F32, tag="sc")
                                for h in range(2):
                                    hh = h * 64
                                    nc.tensor.matmul(
                                        sc[:, h * 512:(h + 1) * 512],
                                        KT[hh:hh + 64, pair,
                                           kc * P:(kc + 1) * P],
                                        QT[hh:hh + 64, pair, q0:q0 + 512],
                                        start=True, stop=True)
                                if kc > 0:
                                    for h in range(2):
                                        nc.tensor.matmul(
                                            accs[h][:],
                                            VP[:, kc - 1, 2 * pair + h, :],
                                            prev_pt[:, h * 512:(h + 1) * 512],
                                            start=(kc == 1), stop=False)
                                it += 1
                                want = (len(fills) * it + nit - 1) // nit
                                while fi < min(want, len(fills)):
                                    fills[fi]()
                                    fi += 1
                                pt = ptp.tile([P, 1024], BF16, tag="pt")
                                nc.scalar.activation(pt[:], sc[:],
                                                     Exp, scale=0.125)
                                prev_pt = pt
                            for h in range(2):
                                nc.tensor.matmul(
                                    accs[h][:],
                                    VP[:, NKC - 1, 2 * pair + h, :],
                                    prev_pt[:, h * 512:(h + 1) * 512],
                                    start=False, stop=True)
                            for h in range(2):
                                hh = h * 64
                                rec = nrmp.tile([1, 512], F32, tag="rec")
                                nc.vector.reciprocal(
                                    rec[:], accs[h][DV:DV + 1, :])
                                bc = nrmp.tile([DV, 512], F32, tag="bc")
                                nc.gpsimd.partition_broadcast(bc[:], rec[:])
                                nc.vector.tensor_mul(
                                    CT[hh:hh + 64, pair, q0:q0 + 512],
                                    accs[h][0:DV, :], bc[:])
                        while fi < len(fills):
                            fills[fi]()
                            fi += 1

            # ---- C: output projection + bias ----
            with (
                tc.tile_pool(name="outp", bufs=2) as outp,
                tc.tile_pool(name="psC", bufs=2,
                             space=bass.MemorySpace.PSUM) as psC,
            ):
                for qb in range(NQB):
                    ops = psC.tile([P, D], F32, tag="o")
                    for pc in range(NPAIR):
                        for half in range(2):
                            nc.tensor.matmul(
                                ops[:, half * 512:(half + 1) * 512],
                                CT[:, pc, qb * P:(qb + 1) * P],
                                wot[:, pc, half * 512:(half + 1) * 512],
                                start=(pc == 0), stop=(pc == NPAIR - 1))
                    outsb = outp.tile([P, D], F32, tag="out")
                    nc.vector.tensor_add(outsb[:], ops[:], bo_bc[:])
                    nc.sync.dma_start(out[qb * P:(qb + 1) * P, :], outsb[:])

    nc.compile()
    return nc


def make_in_maps(x, Wq, Wk, Wv, Wo, bo):
    bf = ml_dtypes.bfloat16
    x = np.asarray(x, np.float32)

    def pm(a):  # [D, N] -> partition-major [P, NDC, N]
        return np.ascontiguousarray(
            a.reshape(NDC, P, a.shape[1]).transpose(1, 0, 2)).astype(bf)

    def wpairs(W):  # [H, D, 64] -> [NPAIR, P, NDC, P]
        a = np.asarray(W, np.float32).transpose(1, 0, 2).reshape(D, H * 64)
        return np.ascontiguousarray(
            a.reshape(NDC, P, NPAIR, P).transpose(2, 1, 0, 3)).astype(bf)

    wk_h = wpairs(Wk)
    wq_h = wpairs(Wq)
    wv_h = pm(np.asarray(Wv, np.float32).transpose(1, 0, 2).reshape(D, H * DV))
    wo_h = np.ascontiguousarray(
        np.asarray(Wo, np.float32).reshape(NPAIR, P, D)
        .transpose(1, 0, 2)).astype(bf)
    bo_h = np.asarray(bo, np.float32).reshape(1, D).astype(bf)

    in_maps = []
    for c in range(NCORES):
        b, qs = c // 2, (c % 2) * NQ
        # Sequence permuted so this core's query half occupies columns
        # 0:NQ (attention is invariant to key order, so K/V computed on
        # the permuted sequence give identical outputs).
        xp = np.concatenate([x[b, qs:qs + NQ], x[b, NQ - qs:2 * NQ - qs]], 0)
        in_maps.append({
            "xT": pm(np.ascontiguousarray(xp.T)),
            "wk": wk_h, "wq": wq_h, "wv": wv_h, "wo": wo_h, "bo": bo_h,
        })
    return in_maps


def kernel(x, Wq, Wk, Wv, Wo, bo):
    from concourse.bass_utils import run_bass_kernel_spmd
    in_maps = make_in_maps(x, Wq, Wk, Wv, Wo, bo)
    nc = build_nc()
    res = run_bass_kernel_spmd(nc, in_maps, list(range(NCORES))).results
    full = np.empty((B, S, D), np.float32)
    for c in range(NCORES):
        b, qs = c // 2, (c % 2) * NQ
        full[b, qs:qs + NQ, :] = np.asarray(res[c]["out"], np.float32)
    return full
